# revision 1
# baseline (speedup 1.0000x reference)
"""ConsistencyLoss kernel for 8 Trainium2 NeuronCores.

Math (per reference):
  For view1: sim = cos_sim_pairwise(y1, z2) [B,N,N]; mask from grid distances;
  loss_v = sum(sim*mask)/sum(mask); out = -(loss_1 + loss_2), N = 28*28 = 784.

Strategy: data-parallel over batch (8 batches/core x 8 cores).
  Host prep (cheap O(B*C*N) work):
    - The grids produced by the reference are exactly separable:
      grid[b,0,i,j] depends only on i, grid[b,1,i,j] only on j.  So the
      pairwise squared distance D2[n,m] = Dy2[i(n),i'(m)] + Dx2[j(n),j'(m)]
      with two tiny [28,28] tables per batch.
    - The mask is a narrow diagonal band: for each 128-row tile of n, every
      masked m lies in a WW=12-image-row window of m whose start the host
      computes from Dy2 (window provably covers all masked pairs).  The
      device only evaluates the [128, 336] window instead of [128, 784].
    - Norms of all four feature tensors; 1/norm of the z-side is folded into
      the z features; 1/norm of the y-side is applied on-device to tiny
      per-tile accumulators (O(N) not O(N^2)).
    - Mask counts (denominators) are computed on host with bit-identical
      fp32 arithmetic to the device mask test.
  Device per batch (n tiled as 6x128+16 partitions):
    - PE: num = y^T @ z_hat windowed (float32r inputs, fp32 PSUM accumulate);
      the moving-operand window offset is a runtime value loaded into a PE
      register (bass.ds dynamic slice).
    - DVE: assemble windowed D2 tiles [128,336] from broadcast APs; fused
      (D2 <= t^2) * num with per-partition accumulation
      (scalar_tensor_tensor); rna-weighted reduction of [128,7] accumulators.
    - Final: partition-reduce via ones-matmul -> [1,2] per-core output.
  Host finish: sum the 8 cores' masked sums, divide by host counts.
"""

import sys

sys.path.insert(0, "/opt/trn_rl_repo")

import numpy as np

import concourse.bass as bass
import concourse.mybir as mybir
import concourse.tile as tile
from concourse import bacc
from concourse.bass import broadcast_tensor_aps
from concourse.bass_utils import run_bass_kernel_spmd

B, C, H, W = 64, 256, 28, 28
N = H * W  # 784
NCORES = 8
BPC = B // NCORES  # batches per core
NT = 7  # n tiles: 6 full 128-partition tiles + one 16-partition tile
NPAD = NT * 128  # 896
THR = 0.7
WW = 12  # window rows (i') per n-tile
WWC = WW * 28  # 336 window columns in m

F32 = mybir.dt.float32
F16 = mybir.dt.float16
F32R = mybir.dt.float32r
I32 = mybir.dt.int32
ALU = mybir.AluOpType
ENG = mybir.EngineType

_COMPILED = {}


def _build_nc():
    nc = bacc.Bacc("TRN2", debug=False, num_devices=NCORES)

    ins = {}
    for nm in ("ay1", "ay2", "bz2", "bz1"):
        ins[nm] = nc.dram_tensor(nm, [BPC, 128, 2, N], F32R, kind="ExternalInput")
    ins["dyw"] = nc.dram_tensor("dyw", [BPC, NT, 128, WW], F32, kind="ExternalInput")
    ins["dx2p"] = nc.dram_tensor("dx2p", [BPC, NPAD, 28], F32, kind="ExternalInput")
    ins["thr"] = nc.dram_tensor("thr", [BPC, 128, 2], F32, kind="ExternalInput")
    ins["rna"] = nc.dram_tensor("rna", [BPC, 128, 2, NT], F32, kind="ExternalInput")
    ins["woff"] = nc.dram_tensor("woff", [BPC, 1, NT], I32, kind="ExternalInput")
    out = nc.dram_tensor("out", [1, 2], F32, kind="ExternalOutput")

    with tile.TileContext(nc) as tc:
        with (
            tc.tile_pool(name="feat", bufs=2) as feat_pool,
            tc.tile_pool(name="dyx", bufs=2) as dyx_pool,
            tc.tile_pool(name="d2", bufs=3) as d2_pool,
            tc.tile_pool(name="scr", bufs=3) as scr_pool,
            tc.tile_pool(name="ms", bufs=2) as ms_pool,
            tc.tile_pool(name="small", bufs=2) as sm_pool,
            tc.tile_pool(name="accum", bufs=1) as acc_pool,
            tc.tile_pool(name="psum", bufs=6, space="PSUM") as psum_pool,
            tc.tile_pool(name="psumf", bufs=1, space="PSUM") as psumf_pool,
        ):
            stot = acc_pool.tile([128, 2, BPC], F32)
            ones_col = acc_pool.tile([128, 1], F32)
            nc.vector.memset(ones_col[:, :], 1.0)

            for b in range(BPC):
                feats = {}
                for nm in ("ay1", "ay2", "bz2", "bz1"):
                    t = feat_pool.tile([128, 2, N], F32R, tag=nm)
                    nc.sync.dma_start(t[:, :, :], ins[nm][b])
                    feats[nm] = t
                dyw_t = dyx_pool.tile([128, NT, WW], F32, tag="dy")
                nc.sync.dma_start(
                    dyw_t[:, :, :], ins["dyw"][b].rearrange("k p w -> p k w")
                )
                dx_t = dyx_pool.tile([128, NT, 28], F32, tag="dx")
                nc.sync.dma_start(
                    dx_t[:, :, :], ins["dx2p"][b].rearrange("(k p) i -> p k i", p=128)
                )
                thr_t = sm_pool.tile([128, 2], F32, tag="thr")
                nc.sync.dma_start(thr_t[:, :], ins["thr"][b])
                rna_t = sm_pool.tile([128, 2, NT], F32, tag="rna")
                nc.sync.dma_start(rna_t[:, :, :], ins["rna"][b])
                woff_t = sm_pool.tile([1, NT], I32, tag="woff")
                nc.sync.dma_start(woff_t[:, :], ins["woff"][b])

                ms = []
                for v in (0, 1):
                    m = ms_pool.tile([128, NT], F32, tag=f"ms{v}")
                    nc.vector.memset(m[:, :], 0.0)
                    ms.append(m)

                for k in range(NT):
                    p = 128 if k < 6 else N - 6 * 128
                    d2 = d2_pool.tile([128, WWC], F32, tag="d2")
                    i0, i1 = broadcast_tensor_aps(
                        dyw_t[:, k, :, None], dx_t[:, k, None, :]
                    )
                    nc.vector.tensor_tensor(
                        d2[:, :].rearrange("q (a c) -> q a c", a=WW), i0, i1, ALU.add
                    )
                    nums = []
                    reg = nc.alloc_registers(
                        name=f"w_{b}_{k}", engines=(ENG.PE,)
                    )
                    nc.tensor.load(reg, woff_t[0:1, k : k + 1])
                    wv = nc.snap(reg, donate=True, min_val=0,
                                 max_val=(28 - WW) * 28)
                    for v, (a_nm, b_nm) in enumerate(
                        (("ay1", "bz2"), ("ay2", "bz1"))
                    ):
                        num = psum_pool.tile([128, WWC], F32, tag="num")
                        a_t, b_t = feats[a_nm], feats[b_nm]
                        for cc in (0, 1):
                            nc.tensor.matmul(
                                num[0:p, :],
                                a_t[:, cc, k * 128 : k * 128 + p],
                                b_t[:, cc, bass.ds(wv, WWC)],
                                start=(cc == 0),
                                stop=(cc == 1),
                            )
                        nums.append(num)
                    for v in (0, 1):
                        scr = scr_pool.tile([128, WWC], F32, tag="scr")
                        nc.vector.scalar_tensor_tensor(
                            out=scr[0:p, :],
                            in0=d2[0:p, :],
                            scalar=thr_t[0:p, v : v + 1],
                            in1=nums[v][0:p, :],
                            op0=ALU.is_le,
                            op1=ALU.mult,
                            accum_out=ms[v][0:p, k : k + 1],
                        )

                for v in (0, 1):
                    wscr = scr_pool.tile([128, NT], F32, tag="wscr")
                    nc.vector.scalar_tensor_tensor(
                        out=wscr[:, :],
                        in0=ms[v][:, :],
                        scalar=1.0,
                        in1=rna_t[:, v, :],
                        op0=ALU.mult,
                        op1=ALU.mult,
                        accum_out=stot[:, v, b : b + 1],
                    )

            sfin = acc_pool.tile([128, 2], F32)
            nc.vector.reduce_sum(sfin[:, :], stot[:, :, :], axis=mybir.AxisListType.X)
            ps = psumf_pool.tile([1, 2], F32)
            nc.tensor.matmul(ps[:, :], ones_col[:, :], sfin[:, :], start=True, stop=True)
            out_s = acc_pool.tile([1, 2], F32)
            nc.vector.tensor_copy(out_s[:, :], ps[:, :])
            nc.sync.dma_start(out[:, :], out_s[:, :])

    nc.compile()
    return nc


def _get_nc():
    if "nc" not in _COMPILED:
        _COMPILED["nc"] = _build_nc()
    return _COMPILED["nc"]


def _prep_host(y1, y2, z1, z2, view1_grid, view2_grid):
    """Host-side prep: separable distance tables, norms, counts, shards."""
    y1f = y1.reshape(B, C, N)
    y2f = y2.reshape(B, C, N)
    z1f = z1.reshape(B, C, N)
    z2f = z2.reshape(B, C, N)

    # --- separable grid tables ------------------------------------------
    g1y = view1_grid[:, 0, :, 0]  # [B, 28] rows (y coordinate per i)
    g1x = view1_grid[:, 1, 0, :]  # [B, 28] cols (x coordinate per j)
    g2y = view2_grid[:, 0, :, 0]
    g2x = view2_grid[:, 1, 0, :]
    if not (
        np.array_equal(view1_grid[:, 0], np.broadcast_to(g1y[:, :, None], (B, H, W)))
        and np.array_equal(view1_grid[:, 1], np.broadcast_to(g1x[:, None, :], (B, H, W)))
        and np.array_equal(view2_grid[:, 0], np.broadcast_to(g2y[:, :, None], (B, H, W)))
        and np.array_equal(view2_grid[:, 1], np.broadcast_to(g2x[:, None, :], (B, H, W)))
    ):
        raise RuntimeError("grids are not separable; unsupported input")

    dy = g1y[:, :, None] - g2y[:, None, :]  # fp32 [B,28,28]
    dx = g1x[:, :, None] - g2x[:, None, :]
    dy2 = dy * dy
    dx2 = dx * dx

    v1bin = np.linalg.norm(
        view1_grid[..., 1, 1] - view1_grid[..., 0, 0], axis=-1
    )  # [B]
    v2bin = np.linalg.norm(view2_grid[..., 1, 1] - view2_grid[..., 0, 0], axis=-1)
    t2 = np.empty((B, 2), np.float32)
    t2[:, 0] = ((THR * v1bin.astype(np.float64)) ** 2).astype(np.float32)
    t2[:, 1] = ((THR * v2bin.astype(np.float64)) ** 2).astype(np.float32)

    # --- per-(batch, tile) windows of valid i' --------------------------
    tmax2 = np.maximum(t2[:, 0], t2[:, 1]).astype(np.float64) * (1 + 1e-6)  # [B]
    w0 = np.zeros((B, NT), np.int32)
    dyw = np.zeros((B, NT, 128, WW), np.float32)
    iidx_all = np.minimum(np.arange(NPAD), N - 1) // 28  # [896]
    for k in range(NT):
        p = 128 if k < 6 else N - 6 * 128
        n0 = 128 * k
        i_lo = n0 // 28
        i_hi = (n0 + p - 1) // 28
        sub_min = dy2[:, i_lo : i_hi + 1, :].min(axis=1)  # [B, 28]
        valid = sub_min <= tmax2[:, None]  # [B, 28]
        any_valid = valid.any(axis=1)
        first = np.argmax(valid, axis=1)
        last = 27 - np.argmax(valid[:, ::-1], axis=1)
        width = np.where(any_valid, last - first + 1, 1)
        if (width > WW).any():
            raise RuntimeError("mask window exceeds WW; unsupported input")
        w0k = np.minimum(np.where(any_valid, first, 0), 28 - WW).astype(np.int32)
        w0[:, k] = w0k
        iidx = iidx_all[n0 : n0 + 128]  # [128]
        cols = w0k[:, None] + np.arange(WW)[None, :]  # [B, WW]
        dyw[:, k] = dy2[
            np.arange(B)[:, None, None], iidx[None, :, None], cols[:, None, :]
        ]
    woff = (w0 * 28).astype(np.int32).reshape(B, 1, NT)

    dx2p = np.zeros((B, NPAD, 28), np.float32)
    dx2p[:, :N] = np.tile(dx2, (1, H, 1))  # row j(n) = n % 28

    # --- mask counts (bit-identical fp32 add + compare as device) -------
    counts = np.zeros(2, np.int64)
    for b in range(B):
        d2b = dy2[b][:, None, :, None] + dx2[b][None, :, None, :]  # fp32
        counts[0] += int((d2b <= t2[b, 0]).sum())
        counts[1] += int((d2b <= t2[b, 1]).sum())

    # --- norms ----------------------------------------------------------
    def rnorm(a):
        n = np.sqrt(np.einsum("bcn,bcn->bn", a, a, dtype=np.float32))
        return 1.0 / np.maximum(n, np.float32(1e-7))

    rna1 = rnorm(y1f)  # y-side view1  [B, N]
    rna2 = rnorm(y2f)  # y-side view2
    rnb1 = rnorm(z2f)  # z-side view1
    rnb2 = rnorm(z1f)  # z-side view2

    def pack_feat(a):
        # [B, C, N] fp -> [B, 128, 2, N] fp32 (float32r on device)
        return np.ascontiguousarray(
            a.reshape(B, 2, 128, N).transpose(0, 2, 1, 3).astype(np.float32)
        )

    ay1 = pack_feat(y1f)
    ay2 = pack_feat(y2f)
    bz2 = pack_feat(z2f * rnb1[:, None, :])
    bz1 = pack_feat(z1f * rnb2[:, None, :])

    # y-side reciprocal norms in [B, 128, 2, NT] layout (zero padded)
    rna = np.zeros((B, 128, 2, NT), np.float32)
    pad1 = np.zeros((B, NPAD), np.float32)
    pad2 = np.zeros((B, NPAD), np.float32)
    pad1[:, :N] = rna1
    pad2[:, :N] = rna2
    rna[:, :, 0, :] = pad1.reshape(B, NT, 128).transpose(0, 2, 1)
    rna[:, :, 1, :] = pad2.reshape(B, NT, 128).transpose(0, 2, 1)

    thr = np.ascontiguousarray(np.broadcast_to(t2[:, None, :], (B, 128, 2)))

    in_maps = []
    for c in range(NCORES):
        s = slice(c * BPC, (c + 1) * BPC)
        in_maps.append(
            {
                "ay1": ay1[s],
                "ay2": ay2[s],
                "bz2": bz2[s],
                "bz1": bz1[s],
                "dyw": np.ascontiguousarray(dyw[s]),
                "dx2p": np.ascontiguousarray(dx2p[s]),
                "thr": thr[s],
                "rna": np.ascontiguousarray(rna[s]),
                "woff": np.ascontiguousarray(woff[s]),
            }
        )
    return in_maps, counts


def kernel(y1, y2, z1, z2, view1_grid, view2_grid):
    y1 = np.asarray(y1, np.float32)
    y2 = np.asarray(y2, np.float32)
    z1 = np.asarray(z1, np.float32)
    z2 = np.asarray(z2, np.float32)
    view1_grid = np.asarray(view1_grid, np.float32)
    view2_grid = np.asarray(view2_grid, np.float32)

    in_maps, counts = _prep_host(y1, y2, z1, z2, view1_grid, view2_grid)
    nc = _get_nc()
    res = run_bass_kernel_spmd(nc, in_maps, core_ids=list(range(NCORES)))
    s = np.zeros(2, np.float64)
    for i in range(NCORES):
        s += res.results[i]["out"][0].astype(np.float64)
    loss = -(
        np.float32(s[0]) / np.float32(counts[0])
        + np.float32(s[1]) / np.float32(counts[1])
    )
    return np.array(loss, dtype=np.float32)



# revision 2
# speedup vs baseline: 1.0059x; 1.0059x over previous
"""ConsistencyLoss kernel for 8 Trainium2 NeuronCores.

Math (per reference):
  For view1: sim = cos_sim_pairwise(y1, z2) [B,N,N]; mask from grid distances;
  loss_v = sum(sim*mask)/sum(mask); out = -(loss_1 + loss_2), N = 28*28 = 784.

Strategy (data-parallel over batch, 8 batches/core x 8 cores):
  Host prep:
    - Grids are separable: pairwise dist^2 D2[n,m] = Dy2[i(n),i'(m)] +
      Dx2[j(n),j'(m)] from two [28,28] tables per batch.
    - n tiled as 7x112 (4 image rows per tile, exact).  The mask band per
      tile fits in a WW-image-row window of m (WW computed from data,
      typically 6 -> 168 moving columns); host computes the window start per
      (batch, tile).
    - 1/||y_n|| folded into y features, 1/||z_m|| folded into z features
      (host, fp32), then features packed to fp16 in one dram tensor
      [BPC, 128, 2(view pair), 2(y|z), 2(c-half), N]; two DMAs per batch so
      view1 compute overlaps view2's feature transfer.
    - Mask counts (denominators) computed on host with bit-identical fp32
      add+compare to the device mask test.
  Device per (batch, tile):
    - Pool/GpSimd: d2 tile [112, WW*28] = dyw ⊕ dxt (broadcast add, SBUF).
    - PE: num_v = y_v^T @ z_v windowed (fp16 in, fp32 PSUM accum over the
      two 128-channel halves); window offset is a runtime PE register
      (bass.ds), 7 offsets loaded per batch in one TensorLoad.
    - DVE: scalar_tensor_tensor (d2 <= thr_v) * num_v with per-partition
      accumulation into ms[v][:, b*7+k].
    - ms accumulators DMA'd raw to HBM at the end (short tail); host does
      the final reduction.
  Host finish: sum the 8 cores' ms, divide by host counts.

The timeline is paced by the feature DMA stream (~36.4us of the ~45us
total); DVE masked-accumulate is the secondary leg (~34us), Pool builds
~24us, PE ~17us.
"""

import sys

sys.path.insert(0, "/opt/trn_rl_repo")

import numpy as np

import concourse.bass as bass
import concourse.mybir as mybir
import concourse.tile as tile
from concourse import bacc
from concourse.bass import broadcast_tensor_aps
from concourse.bass_utils import run_bass_kernel_spmd

B, C, H, W = 64, 256, 28, 28
N = H * W  # 784
NCORES = 8
BPC = B // NCORES  # batches per core
NT = 7  # n tiles of 112 partitions (4 image rows each)
TP = 112  # partitions per tile
THR = 0.7

F32 = mybir.dt.float32
F16 = mybir.dt.float16
I32 = mybir.dt.int32
ALU = mybir.AluOpType
ENG = mybir.EngineType

_COMPILED = {}


def _build_nc(WW=6):
    WWC = WW * 28  # window columns in m
    nc = bacc.Bacc("TRN2", debug=False, num_devices=NCORES)

    # features: [b, p, pr, t, cc, n], pair pr0=(ay1,bz2), pr1=(ay2,bz1)
    feats = nc.dram_tensor("feats", [BPC, 128, 2, 2, 2, N], F16, kind="ExternalInput")
    # small pack: [p(112), b, 7*WW dyw | 28 dxt | 2 thr]
    SX = NT * WW + 28 + 2
    small = nc.dram_tensor("small", [TP, BPC, SX], F32, kind="ExternalInput")
    woff = nc.dram_tensor("woff", [1, BPC * NT], I32, kind="ExternalInput")
    out = nc.dram_tensor("out", [128, 2, BPC * NT], F32, kind="ExternalOutput")

    with tile.TileContext(nc) as tc:
        with (
            tc.tile_pool(name="feat", bufs=4) as feat_pool,
            tc.tile_pool(name="small", bufs=1) as sm_pool,
            tc.tile_pool(name="d2", bufs=16) as d2_pool,
            tc.tile_pool(name="scr", bufs=3) as scr_pool,
            tc.tile_pool(name="accum", bufs=1) as acc_pool,
            tc.tile_pool(name="psum", bufs=6, space="PSUM") as psum_pool,
        ):
            fts = []
            for _ in range(2):
                ft = feat_pool.tile([128, 2, 2, 2, N], F16, tag="ft")
                fts.append(ft)
            nc.sync.dma_start(fts[0][:, 0, :, :, :], feats[0, :, 0])
            sm_t = sm_pool.tile([TP, BPC, SX], F32)
            nc.sync.dma_start(sm_t[:, :, :], small[:, :, :])
            woff_t = sm_pool.tile([1, BPC * NT], I32)
            nc.sync.dma_start(woff_t[:, :], woff[:, :])
            nc.sync.dma_start(fts[0][:, 1, :, :, :], feats[0, :, 1])
            for pr in (0, 1):
                nc.sync.dma_start(fts[1][:, pr, :, :, :], feats[1, :, pr])

            ms = []
            for v in (0, 1):
                m = acc_pool.tile([128, BPC * NT], F32, tag=f"ms{v}")
                nc.vector.memset(m[:, :], 0.0)
                ms.append(m)

            regsets = [
                [nc.alloc_register(ENG.PE, f"w{s}_{k}") for k in range(NT)]
                for s in (0, 1)
            ]

            for b in range(BPC):
                if b < 2:
                    ft = fts[b]
                else:
                    ft = feat_pool.tile([128, 2, 2, 2, N], F16, tag="ft")
                    for pr in (0, 1):
                        nc.sync.dma_start(ft[:, pr, :, :, :], feats[b, :, pr])

                regs = regsets[b % 2]
                nc.tensor.load(regs, woff_t[0:1, b * NT : (b + 1) * NT])
                wvs = [
                    nc.snap(
                        bass.RegisterHandles([regs[k]]),
                        donate=True,
                        min_val=0,
                        max_val=(28 - WW) * 28,
                    )
                    for k in range(NT)
                ]

                d2s = []
                for k in range(NT):
                    d2 = d2_pool.tile([TP, WWC], F32, tag="d2")
                    i0, i1 = broadcast_tensor_aps(
                        sm_t[:, b, k * WW : (k + 1) * WW, None],
                        sm_t[:, b, None, NT * WW : NT * WW + 28],
                    )
                    nc.gpsimd.tensor_tensor(
                        d2[:, :].rearrange("q (a c) -> q a c", a=WW), i0, i1, ALU.add
                    )
                    d2s.append(d2)

                for v in (0, 1):
                    for k in range(NT):
                        num = psum_pool.tile([TP, WWC], F32, tag="num")
                        for cc in (0, 1):
                            nc.tensor.matmul(
                                num[:, :],
                                ft[:, v, 0, cc, k * TP : (k + 1) * TP],
                                ft[:, v, 1, cc, bass.ds(wvs[k], WWC)],
                                start=(cc == 0),
                                stop=(cc == 1),
                            )
                        scr = scr_pool.tile([TP, WWC], F32, tag="scr")
                        nc.vector.scalar_tensor_tensor(
                            out=scr[:, :],
                            in0=d2s[k][:, :],
                            scalar=sm_t[:, b, NT * WW + 28 + v : NT * WW + 29 + v],
                            in1=num[:, :],
                            op0=ALU.is_le,
                            op1=ALU.mult,
                            accum_out=ms[v][0:TP, b * NT + k : b * NT + k + 1],
                        )

            nc.scalar.dma_start(out[:, 0, :], ms[0][:, :])
            nc.sync.dma_start(out[:, 1, :], ms[1][:, :])

    nc.compile()
    return nc


def _get_nc(WW):
    if WW not in _COMPILED:
        _COMPILED[WW] = _build_nc(WW)
    return _COMPILED[WW]


def _prep_host(y1, y2, z1, z2, view1_grid, view2_grid):
    """Host-side prep: separable distance tables, norms, counts, shards."""
    y1f = y1.reshape(B, C, N)
    y2f = y2.reshape(B, C, N)
    z1f = z1.reshape(B, C, N)
    z2f = z2.reshape(B, C, N)

    # --- separable grid tables ------------------------------------------
    g1y = view1_grid[:, 0, :, 0]  # [B, 28]
    g1x = view1_grid[:, 1, 0, :]
    g2y = view2_grid[:, 0, :, 0]
    g2x = view2_grid[:, 1, 0, :]
    if not (
        np.array_equal(view1_grid[:, 0], np.broadcast_to(g1y[:, :, None], (B, H, W)))
        and np.array_equal(view1_grid[:, 1], np.broadcast_to(g1x[:, None, :], (B, H, W)))
        and np.array_equal(view2_grid[:, 0], np.broadcast_to(g2y[:, :, None], (B, H, W)))
        and np.array_equal(view2_grid[:, 1], np.broadcast_to(g2x[:, None, :], (B, H, W)))
    ):
        raise RuntimeError("grids are not separable; unsupported input")

    dy = g1y[:, :, None] - g2y[:, None, :]  # fp32 [B,28,28]
    dx = g1x[:, :, None] - g2x[:, None, :]
    dy2 = dy * dy
    dx2 = dx * dx

    v1bin = np.linalg.norm(view1_grid[..., 1, 1] - view1_grid[..., 0, 0], axis=-1)
    v2bin = np.linalg.norm(view2_grid[..., 1, 1] - view2_grid[..., 0, 0], axis=-1)
    t2 = np.empty((B, 2), np.float32)
    t2[:, 0] = ((THR * v1bin.astype(np.float64)) ** 2).astype(np.float32)
    t2[:, 1] = ((THR * v2bin.astype(np.float64)) ** 2).astype(np.float32)

    # --- per-(batch, tile) windows of valid i' --------------------------
    tmax2 = np.maximum(t2[:, 0], t2[:, 1]).astype(np.float64) * (1 + 1e-6)  # [B]
    w0 = np.zeros((B, NT), np.int32)
    widths = np.zeros((B, NT), np.int64)
    for k in range(NT):
        sub_min = dy2[:, 4 * k : 4 * k + 4, :].min(axis=1)  # [B, 28]
        valid = sub_min <= tmax2[:, None]
        any_valid = valid.any(axis=1)
        first = np.argmax(valid, axis=1)
        last = 27 - np.argmax(valid[:, ::-1], axis=1)
        widths[:, k] = np.where(any_valid, last - first + 1, 1)
        w0[:, k] = np.where(any_valid, first, 0)
    WW = max(6, int(widths.max()))
    if WW > 28:
        raise RuntimeError("mask window exceeds image; unsupported input")
    w0 = np.minimum(w0, 28 - WW).astype(np.int32)

    # dyw[b, p(112), k, a] = dy2[b, 4k + p//28, w0[b,k]+a]
    iidx = np.arange(TP) // 28  # [112] image row within tile
    cols = w0[:, :, None] + np.arange(WW)[None, None, :]  # [B, NT, WW]
    dyw = dy2[
        np.arange(B)[:, None, None, None],
        (iidx[None, :, None, None] + 4 * np.arange(NT)[None, None, :, None]),
        cols[:, None, :, :],
    ]  # [B, 112, NT, WW]
    woff = (w0 * 28).astype(np.int32).reshape(B, NT)

    # dxt[b, p, c] = dx2[b, p%28, c]
    dxt = dx2[:, np.tile(np.arange(28), 4), :]  # [B, 112, 28]

    # --- mask counts (bit-identical fp32 add + compare as device) -------
    counts = np.zeros(2, np.int64)
    for b in range(B):
        d2b = dy2[b][:, None, :, None] + dx2[b][None, :, None, :]  # fp32
        counts[0] += int((d2b <= t2[b, 0]).sum())
        counts[1] += int((d2b <= t2[b, 1]).sum())

    # --- norms (both sides folded on host) ------------------------------
    def rnorm(a):
        n = np.sqrt(np.einsum("bcn,bcn->bn", a, a, dtype=np.float32))
        return 1.0 / np.maximum(n, np.float32(1e-7))

    rna1 = rnorm(y1f)
    rna2 = rnorm(y2f)
    rnb1 = rnorm(z2f)
    rnb2 = rnorm(z1f)

    # feats[b, p, pr, t, cc, n] fp16, (pr,t): (0,0)=ay1 (0,1)=bz2 (1,0)=ay2 (1,1)=bz1
    feats = np.empty((B, 128, 2, 2, 2, N), np.float16)
    for (pr, t), a in (
        ((0, 0), y1f * rna1[:, None, :]),
        ((0, 1), z2f * rnb1[:, None, :]),
        ((1, 0), y2f * rna2[:, None, :]),
        ((1, 1), z1f * rnb2[:, None, :]),
    ):
        feats[:, :, pr, t] = (
            a.reshape(B, 2, 128, N).transpose(0, 2, 1, 3).astype(np.float16)
        )

    # small pack [p(112), b, 7*WW dyw | 28 dxt | 2 thr]
    SX = NT * WW + 28 + 2
    small = np.empty((B, TP, SX), np.float32)
    small[:, :, : NT * WW] = dyw.transpose(0, 1, 2, 3).reshape(B, TP, NT * WW)
    small[:, :, NT * WW : NT * WW + 28] = dxt
    small[:, :, NT * WW + 28 :] = np.broadcast_to(t2[:, None, :], (B, TP, 2))

    in_maps = []
    for c in range(NCORES):
        s = slice(c * BPC, (c + 1) * BPC)
        in_maps.append(
            {
                "feats": np.ascontiguousarray(feats[s]),
                "small": np.ascontiguousarray(small[s].transpose(1, 0, 2)),
                "woff": np.ascontiguousarray(woff[s].reshape(1, BPC * NT)),
            }
        )
    return in_maps, counts, WW


def kernel(y1, y2, z1, z2, view1_grid, view2_grid):
    y1 = np.asarray(y1, np.float32)
    y2 = np.asarray(y2, np.float32)
    z1 = np.asarray(z1, np.float32)
    z2 = np.asarray(z2, np.float32)
    view1_grid = np.asarray(view1_grid, np.float32)
    view2_grid = np.asarray(view2_grid, np.float32)

    in_maps, counts, WW = _prep_host(y1, y2, z1, z2, view1_grid, view2_grid)
    nc = _get_nc(WW)
    res = run_bass_kernel_spmd(nc, in_maps, core_ids=list(range(NCORES)))
    s = np.zeros(2, np.float64)
    for i in range(NCORES):
        o = res.results[i]["out"].astype(np.float64)  # [128, 2, BPC*NT]
        s += o.sum(axis=(0, 2))
    loss = -(
        np.float32(s[0]) / np.float32(counts[0])
        + np.float32(s[1]) / np.float32(counts[1])
    )
    return np.array(loss, dtype=np.float32)


# revision 3
# speedup vs baseline: 1.0127x; 1.0068x over previous
"""ConsistencyLoss kernel for 8 Trainium2 NeuronCores.

Math (per reference):
  For view1: sim = cos_sim_pairwise(y1, z2) [B,N,N]; mask from grid distances;
  loss_v = sum(sim*mask)/sum(mask); out = -(loss_1 + loss_2), N = 28*28 = 784.

Strategy (data-parallel over batch, 8 batches/core x 8 cores):
  Host prep:
    - Grids are separable: pairwise dist^2 D2[n,m] = Dy2[i(n),i'(m)] +
      Dx2[j(n),j'(m)] from two [28,28] tables per batch.
    - n tiled as 7x112 (4 image rows per tile, exact).  The mask band per
      tile fits in a WW-image-row window of m (WW computed from data,
      typically 6 -> 168 moving columns); host computes the window start per
      (batch, tile).
    - 1/||y_n|| folded into y features, 1/||z_m|| folded into z features
      (host, fp32), then features packed to fp16 in one dram tensor
      [BPC, 128, 2(view pair), 2(y|z), 2(c-half), N]; two DMAs per batch so
      view1 compute overlaps view2's feature transfer.
    - Mask counts (denominators) computed on host with bit-identical fp32
      add+compare to the device mask test.
  Device per (batch, tile):
    - Pool/GpSimd: d2 tile [112, WW*28] = dyw ⊕ dxt (broadcast add, SBUF).
    - PE: num_v = y_v^T @ z_v windowed (fp16 in, fp32 PSUM accum over the
      two 128-channel halves); window offset is a runtime PE register
      (bass.ds), 7 offsets loaded per batch in one TensorLoad.
    - DVE: scalar_tensor_tensor (d2 <= thr_v) * num_v with per-partition
      accumulation into ms[v][:, b*7+k].
    - ms accumulators DMA'd raw to HBM at the end (short tail); host does
      the final reduction.
  Host finish: sum the 8 cores' ms, divide by host counts.

The timeline is paced by the feature DMA stream (~36.4us of the ~45us
total); DVE masked-accumulate is the secondary leg (~34us), Pool builds
~24us, PE ~17us.
"""

import sys

sys.path.insert(0, "/opt/trn_rl_repo")

import numpy as np

import concourse.bass as bass
import concourse.mybir as mybir
import concourse.tile as tile
from concourse import bacc
from concourse.bass import broadcast_tensor_aps
from concourse.bass_utils import run_bass_kernel_spmd

B, C, H, W = 64, 256, 28, 28
N = H * W  # 784
NCORES = 8
BPC = B // NCORES  # batches per core
NT = 7  # n tiles of 112 partitions (4 image rows each)
TP = 112  # partitions per tile
THR = 0.7

F32 = mybir.dt.float32
F16 = mybir.dt.float16
I32 = mybir.dt.int32
ALU = mybir.AluOpType
ENG = mybir.EngineType

_COMPILED = {}


def _build_nc(WW=6):
    WWC = WW * 28  # window columns in m
    nc = bacc.Bacc("TRN2", debug=False, num_devices=NCORES)

    # features: [b, p, pr, t, cc, n], pair pr0=(ay1,bz2), pr1=(ay2,bz1)
    feats = nc.dram_tensor("feats", [BPC, 128, 2, 2, 2, N], F16, kind="ExternalInput")
    # small pack: [p(112), b, 7*WW dyw | 28 dxt | 2 thr]
    SX = NT * WW + 28 + 2
    small = nc.dram_tensor("small", [TP, BPC, SX], F32, kind="ExternalInput")
    woff = nc.dram_tensor("woff", [1, BPC * NT], I32, kind="ExternalInput")
    NG = 3  # stt groups per batch (tile triples)
    out = nc.dram_tensor("out", [128, 2, BPC * NG], F32, kind="ExternalOutput")

    with tile.TileContext(nc) as tc:
        with (
            tc.tile_pool(name="feat", bufs=4) as feat_pool,
            tc.tile_pool(name="small", bufs=1) as sm_pool,
            tc.tile_pool(name="d2", bufs=7) as d2_pool,
            tc.tile_pool(name="scr", bufs=3) as scr_pool,
            tc.tile_pool(name="accum", bufs=1) as acc_pool,
            tc.tile_pool(name="psum", bufs=5, space="PSUM") as psum_pool,
        ):
            fts = []
            for _ in range(2):
                ft = feat_pool.tile([128, 2, 2, 2, N], F16, tag="ft")
                fts.append(ft)
            nc.sync.dma_start(fts[0][:, 0, :, :, :], feats[0, :, 0])
            sm_t = sm_pool.tile([TP, BPC, SX], F32)
            nc.sync.dma_start(sm_t[:, :, :], small[:, :, :])
            woff_t = sm_pool.tile([1, BPC * NT], I32)
            nc.sync.dma_start(woff_t[:, :], woff[:, :])
            nc.sync.dma_start(fts[0][:, 1, :, :, :], feats[0, :, 1])
            for pr in (0, 1):
                nc.sync.dma_start(fts[1][:, pr, :, :, :], feats[1, :, pr])

            ms = []
            for v in (0, 1):
                m = acc_pool.tile([128, BPC * NG], F32, tag=f"ms{v}")
                nc.vector.memset(m[:, :], 0.0)
                ms.append(m)

            regsets = [
                [nc.alloc_register(ENG.PE, f"w{s}_{k}") for k in range(NT)]
                for s in (0, 1)
            ]

            for b in range(BPC):
                if b < 2:
                    ft = fts[b]
                else:
                    ft = feat_pool.tile([128, 2, 2, 2, N], F16, tag="ft")
                    for pr in (0, 1):
                        nc.sync.dma_start(ft[:, pr, :, :, :], feats[b, :, pr])

                regs = regsets[b % 2]
                nc.tensor.load(regs, woff_t[0:1, b * NT : (b + 1) * NT])
                wvs = [
                    nc.snap(
                        bass.RegisterHandles([regs[k]]),
                        donate=True,
                        min_val=0,
                        max_val=(28 - WW) * 28,
                    )
                    for k in range(NT)
                ]

                # tile triples share one PSUM bank (3*168 fp32 = 2016B <= 2KB)
                # and one d2 tile, so each masked-accumulate is a single wide
                # stt instead of three.
                GROUPS = ((0, 1, 2), (3, 4, 5), (6,))
                d2s = []
                for g, ks in enumerate(GROUPS):
                    gw = len(ks)
                    d2 = d2_pool.tile([TP, 3, WWC], F32, tag="d2")
                    i0, i1 = broadcast_tensor_aps(
                        sm_t[:, b, ks[0] * WW : (ks[0] + gw) * WW, None],
                        sm_t[:, b, None, NT * WW : NT * WW + 28],
                    )
                    nc.gpsimd.tensor_tensor(
                        d2[:, 0:gw, :].rearrange("q g (a c) -> q (g a) c", a=WW),
                        i0,
                        i1,
                        ALU.add,
                    )
                    d2s.append(d2)

                for v in (0, 1):
                    for g, ks in enumerate(GROUPS):
                        gw = len(ks)
                        num = psum_pool.tile([TP, 3, WWC], F32, tag="num")
                        for j, k in enumerate(ks):
                            for cc in (0, 1):
                                nc.tensor.matmul(
                                    num[:, j, :],
                                    ft[:, v, 0, cc, k * TP : (k + 1) * TP],
                                    ft[:, v, 1, cc, bass.ds(wvs[k], WWC)],
                                    start=(cc == 0),
                                    stop=(cc == 1),
                                )
                        scr = scr_pool.tile([TP, 3 * WWC], F32, tag="scr")
                        col = b * NG + g
                        nc.vector.scalar_tensor_tensor(
                            out=scr[:, 0 : gw * WWC],
                            in0=d2s[g][:, 0:gw, :],
                            scalar=sm_t[:, b, NT * WW + 28 + v : NT * WW + 29 + v],
                            in1=num[:, 0:gw, :],
                            op0=ALU.is_le,
                            op1=ALU.mult,
                            accum_out=ms[v][0:TP, col : col + 1],
                        )

            nc.scalar.dma_start(out[:, 0, :], ms[0][:, :])
            nc.sync.dma_start(out[:, 1, :], ms[1][:, :])

    nc.compile()
    return nc


def _get_nc(WW):
    if WW not in _COMPILED:
        _COMPILED[WW] = _build_nc(WW)
    return _COMPILED[WW]


def _prep_host(y1, y2, z1, z2, view1_grid, view2_grid):
    """Host-side prep: separable distance tables, norms, counts, shards."""
    y1f = y1.reshape(B, C, N)
    y2f = y2.reshape(B, C, N)
    z1f = z1.reshape(B, C, N)
    z2f = z2.reshape(B, C, N)

    # --- separable grid tables ------------------------------------------
    g1y = view1_grid[:, 0, :, 0]  # [B, 28]
    g1x = view1_grid[:, 1, 0, :]
    g2y = view2_grid[:, 0, :, 0]
    g2x = view2_grid[:, 1, 0, :]
    if not (
        np.array_equal(view1_grid[:, 0], np.broadcast_to(g1y[:, :, None], (B, H, W)))
        and np.array_equal(view1_grid[:, 1], np.broadcast_to(g1x[:, None, :], (B, H, W)))
        and np.array_equal(view2_grid[:, 0], np.broadcast_to(g2y[:, :, None], (B, H, W)))
        and np.array_equal(view2_grid[:, 1], np.broadcast_to(g2x[:, None, :], (B, H, W)))
    ):
        raise RuntimeError("grids are not separable; unsupported input")

    dy = g1y[:, :, None] - g2y[:, None, :]  # fp32 [B,28,28]
    dx = g1x[:, :, None] - g2x[:, None, :]
    dy2 = dy * dy
    dx2 = dx * dx

    v1bin = np.linalg.norm(view1_grid[..., 1, 1] - view1_grid[..., 0, 0], axis=-1)
    v2bin = np.linalg.norm(view2_grid[..., 1, 1] - view2_grid[..., 0, 0], axis=-1)
    t2 = np.empty((B, 2), np.float32)
    t2[:, 0] = ((THR * v1bin.astype(np.float64)) ** 2).astype(np.float32)
    t2[:, 1] = ((THR * v2bin.astype(np.float64)) ** 2).astype(np.float32)

    # --- per-(batch, tile) windows of valid i' --------------------------
    tmax2 = np.maximum(t2[:, 0], t2[:, 1]).astype(np.float64) * (1 + 1e-6)  # [B]
    w0 = np.zeros((B, NT), np.int32)
    widths = np.zeros((B, NT), np.int64)
    for k in range(NT):
        sub_min = dy2[:, 4 * k : 4 * k + 4, :].min(axis=1)  # [B, 28]
        valid = sub_min <= tmax2[:, None]
        any_valid = valid.any(axis=1)
        first = np.argmax(valid, axis=1)
        last = 27 - np.argmax(valid[:, ::-1], axis=1)
        widths[:, k] = np.where(any_valid, last - first + 1, 1)
        w0[:, k] = np.where(any_valid, first, 0)
    WW = max(6, int(widths.max()))
    if WW > 28:
        raise RuntimeError("mask window exceeds image; unsupported input")
    w0 = np.minimum(w0, 28 - WW).astype(np.int32)

    # dyw[b, p(112), k, a] = dy2[b, 4k + p//28, w0[b,k]+a]
    iidx = np.arange(TP) // 28  # [112] image row within tile
    cols = w0[:, :, None] + np.arange(WW)[None, None, :]  # [B, NT, WW]
    dyw = dy2[
        np.arange(B)[:, None, None, None],
        (iidx[None, :, None, None] + 4 * np.arange(NT)[None, None, :, None]),
        cols[:, None, :, :],
    ]  # [B, 112, NT, WW]
    woff = (w0 * 28).astype(np.int32).reshape(B, NT)

    # dxt[b, p, c] = dx2[b, p%28, c]
    dxt = dx2[:, np.tile(np.arange(28), 4), :]  # [B, 112, 28]

    # --- mask counts (bit-identical fp32 add + compare as device) -------
    counts = np.zeros(2, np.int64)
    for b in range(B):
        d2b = dy2[b][:, None, :, None] + dx2[b][None, :, None, :]  # fp32
        counts[0] += int((d2b <= t2[b, 0]).sum())
        counts[1] += int((d2b <= t2[b, 1]).sum())

    # --- norms (both sides folded on host) ------------------------------
    def rnorm(a):
        n = np.sqrt(np.einsum("bcn,bcn->bn", a, a, dtype=np.float32))
        return 1.0 / np.maximum(n, np.float32(1e-7))

    rna1 = rnorm(y1f)
    rna2 = rnorm(y2f)
    rnb1 = rnorm(z2f)
    rnb2 = rnorm(z1f)

    # feats[b, p, pr, t, cc, n] fp16, (pr,t): (0,0)=ay1 (0,1)=bz2 (1,0)=ay2 (1,1)=bz1
    feats = np.empty((B, 128, 2, 2, 2, N), np.float16)
    for (pr, t), a in (
        ((0, 0), y1f * rna1[:, None, :]),
        ((0, 1), z2f * rnb1[:, None, :]),
        ((1, 0), y2f * rna2[:, None, :]),
        ((1, 1), z1f * rnb2[:, None, :]),
    ):
        feats[:, :, pr, t] = (
            a.reshape(B, 2, 128, N).transpose(0, 2, 1, 3).astype(np.float16)
        )

    # small pack [p(112), b, 7*WW dyw | 28 dxt | 2 thr]
    SX = NT * WW + 28 + 2
    small = np.empty((B, TP, SX), np.float32)
    small[:, :, : NT * WW] = dyw.transpose(0, 1, 2, 3).reshape(B, TP, NT * WW)
    small[:, :, NT * WW : NT * WW + 28] = dxt
    small[:, :, NT * WW + 28 :] = np.broadcast_to(t2[:, None, :], (B, TP, 2))

    in_maps = []
    for c in range(NCORES):
        s = slice(c * BPC, (c + 1) * BPC)
        in_maps.append(
            {
                "feats": np.ascontiguousarray(feats[s]),
                "small": np.ascontiguousarray(small[s].transpose(1, 0, 2)),
                "woff": np.ascontiguousarray(woff[s].reshape(1, BPC * NT)),
            }
        )
    return in_maps, counts, WW


def kernel(y1, y2, z1, z2, view1_grid, view2_grid):
    y1 = np.asarray(y1, np.float32)
    y2 = np.asarray(y2, np.float32)
    z1 = np.asarray(z1, np.float32)
    z2 = np.asarray(z2, np.float32)
    view1_grid = np.asarray(view1_grid, np.float32)
    view2_grid = np.asarray(view2_grid, np.float32)

    in_maps, counts, WW = _prep_host(y1, y2, z1, z2, view1_grid, view2_grid)
    nc = _get_nc(WW)
    res = run_bass_kernel_spmd(nc, in_maps, core_ids=list(range(NCORES)))
    s = np.zeros(2, np.float64)
    for i in range(NCORES):
        o = res.results[i]["out"].astype(np.float64)  # [128, 2, BPC*NT]
        s += o.sum(axis=(0, 2))
    loss = -(
        np.float32(s[0]) / np.float32(counts[0])
        + np.float32(s[1]) / np.float32(counts[1])
    )
    return np.array(loss, dtype=np.float32)
